# revision 40
# baseline (speedup 1.0000x reference)
"""TRN2 Bass kernel for nn_CustomQLoRABigNet: 6 blocks x (3 QLoRA linears),
ReLU, residual, LayerNorm. Data-parallel over 8 NeuronCores (4096 rows each).

v2 strategy vs baseline:
- LoRA is folded into the dequantized weight once per layer:
  W_eff^T = (q-8)*s + la^T @ lb^T  (16 contraction-32 matmuls + vector adds),
  eliminating the per-activation-tile LoRA stage1/stage2 matmul streams.
- Everything on-chip is bf16 (fp32 PSUM accumulation), halving SBUF/DMA and
  enabling fast weight loads; hidden state is a single full-width buffer
  [128, 8, 4096] updated in place via per-strip snapshots.
- Residual add is fused into the PSUM evacuation on the vector engine
  (scalar_tensor_tensor: (psum + bias) + r); residual tensors are staged
  through DRAM scratch instead of occupying SBUF.
- Weights are built once per layer (single pass over rows), so dequant DMA
  drops from 360MB to ~72MB per core.
"""

import sys

sys.path.insert(0, "/opt/trn_rl_repo")

import numpy as np
import ml_dtypes

import concourse.bass as bass
from concourse import bacc, mybir
import concourse.tile as tile
from concourse.bass_utils import run_bass_kernel_spmd

f32 = mybir.dt.float32
f32r = mybir.dt.float32r
bf16 = mybir.dt.bfloat16
AF = mybir.ActivationFunctionType
Alu = mybir.AluOpType
BF = ml_dtypes.bfloat16

N_CORES = 8
DIM = 1024
KT = 8  # 1024 / 128 partition tiles
NL = 18
RANK = 32
GROUP = 16
BATCH = 32768
RPC = BATCH // N_CORES  # rows per core
NT = 512  # matmul moving free dim (one PSUM bank of fp32)
EPS = 1e-5

# Weight/operand dtype mode: True = bf16 weights (single-rounding build) and
# bf16 moving operand; False = f32r weights + f32r snapshot (more accurate,
# slower weight loads). Both keep f32 scales and f32 LN normalizers.
W_BF16 = True


def build_kernel(rows: int = RPC, n_layers: int = NL):
    nc = bacc.Bacc()
    nstrip = rows // NT
    n_blocks = n_layers // 3

    x_d = nc.declare_dram_parameter("x_t", [128, KT, rows], bf16, False)
    wq_d = nc.declare_dram_parameter("wq_b", [n_layers, 128, KT, DIM], bf16, False)
    sr_d = nc.declare_dram_parameter("srep", [n_layers, 128, KT, DIM], f32, False)
    # la packed for 2-way PE row-group concurrency: [l, 32g+r, i, kp] holds
    # la[l, r, (2i+g)*128+kp]; lb^T replicated into both 32-partition groups
    la_d = nc.declare_dram_parameter("la_f", [n_layers, 2 * RANK, KT // 2, 128], bf16, False)
    lb_d = nc.declare_dram_parameter("lb_f", [n_layers, 2 * RANK, DIM], bf16, False)
    bi_d = nc.declare_dram_parameter("bias_pp", [128, n_layers, KT], f32, False)
    ga_d = nc.declare_dram_parameter("gamma_pp", [128, 5, KT], f32, False)
    be_d = nc.declare_dram_parameter("beta_pp", [128, 5, KT], f32, False)
    on_d = nc.declare_dram_parameter("ones", [128, 128], bf16, False)
    onf_d = nc.declare_dram_parameter("ones_f", [1, 128], f32r, False)
    y_d = nc.declare_dram_parameter("y_t", [128, KT, rows], bf16, True)

    with tile.TileContext(nc) as tc:
        with (
            tc.tile_pool(name="persist", bufs=1) as pp,
            tc.tile_pool(name="wts", bufs=2) as wp,
            tc.tile_pool(name="stage", bufs=2) as hp,
            tc.tile_pool(name="small", bufs=2) as sp,
            tc.tile_pool(name="ps_y", bufs=5, space="PSUM") as psy,
            tc.tile_pool(name="ps_f", bufs=3, space="PSUM") as psf,
            tc.tile_pool(name="rdram", bufs=1, space="DRAM") as dr,
        ):
            h_t = pp.tile([128, KT, rows], bf16)
            bias_t = pp.tile([128, n_layers, KT], f32)
            nc.sync.dma_start(bias_t[:, :, :], bi_d[:, :, :])
            gamma_t = pp.tile([128, 5, KT], f32)
            nc.sync.dma_start(gamma_t[:, :, :], ga_d[:, :, :])
            beta_t = pp.tile([128, 5, KT], f32)
            nc.sync.dma_start(beta_t[:, :, :], be_d[:, :, :])
            ones_t = pp.tile([128, 128], bf16)
            nc.sync.dma_start(ones_t[:, :], on_d[:, :])
            ones_col = ones_t[:, 0:1]
            ones_fr = pp.tile([1, 128], f32r)
            nc.sync.dma_start(ones_fr[:, :], onf_d[:, :])
            ones_row = ones_fr[0:1, :]

            # residual ping-pong scratch in DRAM (block b reads r_dram[b%2],
            # its LayerNorm output is written to r_dram[(b+1)%2])
            r_dram = [
                dr.tile([128, KT, rows], bf16, tag=f"r{i}", name=f"r_dram{i}")
                for i in range(2)
            ]

            nc.sync.dma_start(h_t[:, :, :], x_d[:, :, :])

            def build_weights(l):
                """w_eff(l) = (q-8)*s + la^T @ lb^T"""
                w_t = wp.tile(
                    [128, KT, DIM], bf16 if W_BF16 else f32r, tag="we",
                    name=f"we{l}", bufs=3,
                )
                la_t = wp.tile([2 * RANK, KT // 2, 128], bf16, tag="la", name=f"la{l}")
                nc.sync.dma_start(la_t[:, :, :], la_d[l, :, :, :])
                lb_t = wp.tile([2 * RANK, DIM], bf16, tag="lb", name=f"lb{l}")
                nc.sync.dma_start(lb_t[:, :], lb_d[l, :, :])
                for i in range(KT // 2):
                    dqs = []
                    for g in range(2):
                        kt = 2 * i + g
                        wq_t = wp.tile([128, DIM], bf16, tag="wq", name=f"wq{l}_{kt}")
                        nc.sync.dma_start(wq_t[:, :], wq_d[l, :, kt, :])
                        sr_t = wp.tile([128, DIM], f32, tag="sr", name=f"sr{l}_{kt}")
                        nc.sync.dma_start(sr_t[:, :], sr_d[l, :, kt, :])
                        if W_BF16:
                            # keep the product in f32: w_eff rounds only once
                            wt_f = wp.tile(
                                [128, DIM], f32, tag="wtf", name=f"wf{l}_{kt}"
                            )
                            nc.vector.tensor_mul(wt_f[:, :], wq_t[:, :], sr_t[:, :])
                            dqs.append(wt_f)
                        else:
                            nc.vector.tensor_mul(
                                w_t[:, kt, :], wq_t[:, :], sr_t[:, :]
                            )
                            dqs.append(w_t[:, kt, :])
                    for oh in range(2):
                        ohc = bass.ts(oh, NT)
                        fps = []
                        # two fold matmuls run concurrently in 32-row PE groups
                        for g in range(2):
                            f_ps = psf.tile(
                                [128, NT], f32, tag="fold", name=f"fps{l}_{i}_{oh}_{g}"
                            )
                            gp = slice(RANK * g, RANK * (g + 1))
                            nc.tensor.matmul(
                                f_ps[:, :],
                                lhsT=la_t[gp, i, :],
                                rhs=lb_t[gp, ohc],
                                start=True,
                                stop=True,
                                tile_position=(RANK * g, 0),
                            )
                            fps.append(f_ps)
                        for g in range(2):
                            kt = 2 * i + g
                            nc.vector.tensor_add(
                                w_t[:, kt, ohc],
                                dqs[g][:, ohc] if W_BF16 else w_t[:, kt, ohc],
                                fps[g][:, :],
                            )
                return w_t

            w_tiles = {0: build_weights(0), 1: build_weights(1)}

            for l in range(n_layers):
                blk, j = l // 3, l % 3
                ln_here = j == 2 and blk < n_blocks - 1
                w_t = w_tiles.pop(l)

                # ---- main pass: h[:, :, strip] = layer(h[:, :, strip]) ----
                for s in range(nstrip):
                    scols = bass.ts(s, NT)
                    # snapshot enables in-place h update; in f32r mode it also
                    # converts bf16 -> f32r (matmul operand classes must match)
                    hs = hp.tile([128, KT, NT], bf16 if W_BF16 else f32r, tag="hs")
                    nc.vector.tensor_copy(hs[:, :, :], h_t[:, :, scols])
                    if j == 2:
                        r_st = hp.tile([128, KT, NT], bf16, tag="rst", bufs=1)
                        if blk == 0:
                            nc.sync.dma_start(r_st[:, :, :], x_d[:, :, scols])
                        else:
                            nc.sync.dma_start(
                                r_st[:, :, :], r_dram[blk % 2][:, :, scols]
                            )
                    for ot in range(KT):
                        y_ps = psy.tile([128, NT], f32, tag="y")
                        for kt in range(KT):
                            nc.tensor.matmul(
                                y_ps[:, :],
                                lhsT=w_t[:, kt, bass.ts(ot, 128)],
                                rhs=hs[:, kt, :],
                                start=(kt == 0),
                                stop=(kt == KT - 1),
                            )
                        if j < 2:
                            if ot % 2 == 0:
                                nc.scalar.activation(
                                    h_t[:, ot, scols],
                                    y_ps[:, :],
                                    AF.Relu,
                                    bias=bias_t[:, l, ot : ot + 1],
                                )
                            else:
                                # relu(y + bias) on vector: (y add b) max 0
                                nc.vector.tensor_scalar(
                                    h_t[:, ot, scols],
                                    y_ps[:, :],
                                    bias_t[:, l, ot : ot + 1],
                                    0.0,
                                    Alu.add,
                                    Alu.max,
                                )
                        else:
                            # h = (psum + bias) + r fused: one bf16 rounding
                            nc.vector.scalar_tensor_tensor(
                                h_t[:, ot, scols],
                                y_ps[:, :],
                                bias_t[:, l, ot : ot + 1],
                                r_st[:, ot, :],
                                Alu.add,
                                Alu.add,
                            )

                    # ---- LayerNorm at block end (blocks 0..4) ----
                    if ln_here:
                        s1p = psf.tile([1, NT], f32, tag="fold", name="s1p")
                        s2p = psf.tile([1, NT], f32, tag="fold", name="s2p")
                        for ot in range(KT):
                            hsq = sp.tile([128, NT], bf16, tag="hsq", bufs=1)
                            nc.scalar.activation(
                                hsq[:, :], h_t[:, ot, scols], AF.Square
                            )
                            nc.tensor.matmul(
                                s1p[:, :], lhsT=ones_col, rhs=h_t[:, ot, scols],
                                start=(ot == 0), stop=(ot == KT - 1),
                            )
                            nc.tensor.matmul(
                                s2p[:, :], lhsT=ones_col, rhs=hsq[:, :],
                                start=(ot == 0), stop=(ot == KT - 1),
                            )
                        m_sb = sp.tile([1, NT], f32, tag="m", bufs=1)
                        nc.vector.tensor_scalar(
                            m_sb[:, :], s1p[:, :], 1.0 / DIM, None, Alu.mult
                        )
                        msq = sp.tile([1, NT], f32, tag="msq", bufs=1)
                        nc.vector.tensor_mul(msq[:, :], m_sb[:, :], m_sb[:, :])
                        # var = s2/D - m^2  (eps dropped: var >> 1e-5 here,
                        # relative effect < 1e-5 on the normalizer)
                        var_sb = sp.tile([1, NT], f32, tag="var", bufs=1)
                        nc.vector.scalar_tensor_tensor(
                            var_sb[:, :], s2p[:, :], 1.0 / DIM, msq[:, :],
                            Alu.mult, Alu.subtract,
                        )
                        lnv = sp.tile([1, NT], f32, tag="lnv", bufs=1)
                        nc.scalar.activation(lnv[:, :], var_sb[:, :], AF.Ln)
                        i_sb = sp.tile([1, NT], f32r, tag="isb", bufs=1)
                        nc.scalar.activation(i_sb[:, :], lnv[:, :], AF.Exp, scale=-0.5)
                        mi_sb = sp.tile([1, NT], f32r, tag="misb", bufs=1)
                        nc.vector.tensor_mul(mi_sb[:, :], m_sb[:, :], i_sb[:, :])
                        ib_ps = psf.tile([128, NT], f32, tag="fold", name="ibps")
                        nc.tensor.matmul(
                            ib_ps[:, :], lhsT=ones_row, rhs=i_sb[:, :],
                            start=True, stop=True,
                        )
                        mib_ps = psf.tile([128, NT], f32, tag="fold", name="ibps")
                        nc.tensor.matmul(
                            mib_ps[:, :], lhsT=ones_row, rhs=mi_sb[:, :],
                            start=True, stop=True,
                        )
                        # evacuate broadcasts to SBUF (f32r) so the apply ops
                        # stay off the PSUM fabric while PE streams
                        ib_sb = sp.tile([128, NT], f32r, tag="ibsb")
                        nc.scalar.activation(ib_sb[:, :], ib_ps[:, :], AF.Copy)
                        mib_sb = sp.tile([128, NT], f32r, tag="mibsb")
                        nc.scalar.activation(mib_sb[:, :], mib_ps[:, :], AF.Copy)
                        for kt in range(KT):
                            # single-rounding LayerNorm apply
                            tmp = sp.tile([128, NT], f32, tag="lntmp", bufs=1)
                            nc.vector.tensor_mul(
                                tmp[:, :], h_t[:, kt, scols], ib_sb[:, :]
                            )
                            nc.vector.tensor_sub(
                                h_t[:, kt, scols], tmp[:, :], mib_sb[:, :]
                            )
                            nc.scalar.activation(
                                h_t[:, kt, scols],
                                h_t[:, kt, scols],
                                AF.Identity,
                                bias=beta_t[:, blk, kt : kt + 1],
                                scale=gamma_t[:, blk, kt : kt + 1],
                            )
                        nc.sync.dma_start(
                            r_dram[(blk + 1) % 2][:, :, scols], h_t[:, :, scols]
                        )
                    if l == n_layers - 1:
                        nc.sync.dma_start(y_d[:, :, scols], h_t[:, :, scols])

                # build two layers ahead so fold matmuls/adds interleave with
                # this layer's stream instead of bunching at the boundary
                if l + 2 < n_layers:
                    w_tiles[l + 2] = build_weights(l + 2)

    nc.compile()
    return nc


def prep_inputs(x, wq, scales, bias, lora_a, lora_b, gamma, beta,
                rows_per_core=RPC, n_layers=NL):
    """Host-side pure layout/cast prep; returns per-core input maps."""
    nl = n_layers
    # centered transposed weights: [l, p, kt, o] with k = kt*128 + p
    wqc = (wq[:nl].transpose(0, 2, 1).astype(np.float32) - 8.0)
    wqc = wqc.reshape(nl, KT, 128, DIM).transpose(0, 2, 1, 3).astype(BF).copy()

    # per-group scales replicated to the same [l, p, kt, o] layout
    G = scales[:nl].reshape(nl, DIM, 64)  # [l, o, kgroup]
    p_idx = np.arange(128)[:, None] // GROUP  # [128,1]
    kt_idx = np.arange(KT)[None, :] * (128 // GROUP)  # [1,8]
    gidx = p_idx + kt_idx  # [128, 8]
    srep = G.transpose(0, 2, 1)[:, gidx, :].astype(np.float32).copy()  # [l,128,8,o]

    # pack la for 2-way PE row-group concurrency: [l, 32g+r, i, kp] =
    # la[l, r, (2i+g)*128+kp]; replicate lb^T into both 32-row groups
    la4 = lora_a[:nl].reshape(nl, RANK, KT // 2, 2, 128)  # [l, r, i, g, kp]
    la_f = np.ascontiguousarray(
        la4.transpose(0, 3, 1, 2, 4).reshape(nl, 2 * RANK, KT // 2, 128)
    ).astype(BF)
    lbt = lora_b[:nl].transpose(0, 2, 1)  # [l, r, o]
    lb_f = np.concatenate([lbt, lbt], axis=1).astype(BF).copy()  # [l, 2r, o]

    bias_pp = bias[:nl].reshape(nl, KT, 128).transpose(2, 0, 1).astype(np.float32).copy()
    gamma_pp = gamma.reshape(5, KT, 128).transpose(2, 0, 1).astype(np.float32).copy()
    beta_pp = beta.reshape(5, KT, 128).transpose(2, 0, 1).astype(np.float32).copy()

    shared = {
        "wq_b": wqc, "srep": srep, "la_f": la_f, "lb_f": lb_f,
        "bias_pp": bias_pp, "gamma_pp": gamma_pp, "beta_pp": beta_pp,
        "ones": np.ones((128, 128), BF),
        "ones_f": np.ones((1, 128), np.float32),
    }
    in_maps = []
    for c in range(x.shape[0] // rows_per_core):
        xs = x[c * rows_per_core : (c + 1) * rows_per_core]  # [rows, 1024]
        x_t = xs.T.reshape(KT, 128, rows_per_core).transpose(1, 0, 2).astype(BF).copy()
        in_maps.append({"x_t": x_t, **shared})
    return in_maps


def unshard_output(results, rows_per_core=RPC):
    outs = []
    for r in results:
        y_t = np.asarray(r["y_t"]).reshape(128, KT, rows_per_core)
        outs.append(y_t.transpose(2, 1, 0).reshape(rows_per_core, DIM))
    return np.ascontiguousarray(np.concatenate(outs, axis=0), dtype=np.float32)


def kernel(x, wq, scales, bias, lora_a, lora_b, gamma, beta):
    x, wq, scales, bias, lora_a, lora_b, gamma, beta = (
        np.asarray(a) for a in (x, wq, scales, bias, lora_a, lora_b, gamma, beta)
    )
    nc = build_kernel()
    in_maps = prep_inputs(x, wq, scales, bias, lora_a, lora_b, gamma, beta)
    res = run_bass_kernel_spmd(nc, in_maps, list(range(N_CORES)))
    return unshard_output(res.results)
